# revision 19
# baseline (speedup 1.0000x reference)
"""Trainium2 Bass kernel for nn_GRUModel (segment-GRU encoder + 1-step GRU decoder).

Sharding: data-parallel over batch B: 8 cores x 16 batches each
(rows n = b_loc*64 + c, R=1024 rows/core). Weights replicated.

v5 mixed precision (validated in numpy at rel_err 1.29e-2 vs 2e-2 gate):
- r/z-gate x-side matmuls: fp8e4 DoubleRow (K=256/instr, halves the
  instruction count; each LDWEIGHTS+MATMUL pair costs ~222ns regardless
  of dtype, so fewer instructions = faster). Weights upscaled x8 to
  clear e4m3's subnormal floor; the 1/8 rides the ACT drain's scale.
  The h-side weights of the same PSUM chains are scaled x8 in bf16
  (exact), so one scale covers the whole pre-activation.
- xn (tanh-sensitive), h-side, res, decoder, pred: bf16.
- emb: f32r matmul (full bf16-speed at N=512) with K=66: row 64 folds
  -seq_last, row 65 (ones) folds b_emb. Kills the f32->bf16 cast and
  the per-chunk biased sigmoid: emb sigmoid is ONE [128,2048] ACT op.
- embB (bf16, from the silu tt) feeds h composition CLEAN of fp8 noise;
  embT8 = fp8 copy (ACT Copy) only feeds the rz DR matmuls.
- n-gate: ACT Tanh directly (sigmoid_and_others table holds
  sigmoid+tanh+identity: zero table swaps).

Elementwise batching: t2/tanh/hc run as [128, 2048] ops (t1 and
hT_new stay per-chunk: their per-partition biases differ per chunk).
PSUM: "ps" [128,512] x4 rotating + "pb" [128,2048] x1 (emb then res
per step, program-ordered).

Schedule skeleton (from the tuned baseline): halves A/B of C=512 rows;
encoder-A then encoder-B with decoder-A interleaved (DVE-broadcast
path), decoder-B in the tail via PE-assisted select-matmul path.
"""
import numpy as np
import ml_dtypes

import concourse.bass as bass
import concourse.bacc as bacc
import concourse.mybir as mybir
from concourse import tile
from concourse.bass_utils import run_bass_kernel_spmd

bf16 = ml_dtypes.bfloat16
f8 = ml_dtypes.float8_e4m3
F32 = mybir.dt.float32
F32R = mybir.dt.float32r
BF16 = mybir.dt.bfloat16
F8 = mybir.dt.float8e4
AF = mybir.ActivationFunctionType
ALU = mybir.AluOpType
DR = mybir.MatmulPerfMode.DoubleRow

B, SEQ, ENC = 128, 1024, 64
D, SEG = 512, 64
SNX = SEQ // SEG          # 16
PRED = 512
SNY = PRED // SEG         # 8
NCORES = 8
BL = B // NCORES          # 16 batches per core
R = BL * ENC              # 1024 rows per core
C = R // 2                # 512 rows per half
BH = BL // 2              # 8 batches per half
KC = D // 128             # 4 contraction chunks
KP = KC // 2              # 2 DoubleRow kc-pairs
G3 = 3 * D
MC = G3 // 128            # 12 gate chunks
XS = 8.0                  # x-side rz upscale

# bias column map
BC_RZ, BC_HN, BC_XN, BC_RES, BC_GBHH, BC_PRED = 0, 8, 12, 16, 20, 32

_PROGRAM = None


def _build_program():
    nc = bacc.Bacc("TRN2", target_bir_lowering=False, debug=False, num_devices=8)
    x_d = nc.dram_tensor("x", [BL, SEQ, ENC], F32, kind="ExternalInput")
    lastrow_d = nc.dram_tensor("lastrow", [1, R], F32, kind="ExternalInput")
    ones_d = nc.dram_tensor("ones", [1, R], F32, kind="ExternalInput")
    wemb_d = nc.dram_tensor("wemb", [66, D], BF16, kind="ExternalInput")
    wx8_d = nc.dram_tensor("wx8", [128, 8 * 512], F8, kind="ExternalInput")
    wxn_d = nc.dram_tensor("wxn", [D, D], BF16, kind="ExternalInput")
    wh8_d = nc.dram_tensor("wh8", [128, 8 * 512], F8, kind="ExternalInput")
    whn_d = nc.dram_tensor("whn", [D, D], BF16, kind="ExternalInput")
    wres_d = nc.dram_tensor("wres", [D, D], BF16, kind="ExternalInput")
    whd_d = nc.dram_tensor("whd", [D, G3], BF16, kind="ExternalInput")
    wpred_d = nc.dram_tensor("wpred", [D, SEG], BF16, kind="ExternalInput")
    gxd_d = nc.dram_tensor("gxd", [128, MC * SNY * ENC], BF16, kind="ExternalInput")
    gxdt_d = nc.dram_tensor("gxdt", [128, SNY * 6 * 128], BF16, kind="ExternalInput")
    ident_d = nc.dram_tensor("ident", [128, 128], BF16, kind="ExternalInput")
    selm_d = nc.dram_tensor("selm", [128, C], BF16, kind="ExternalInput")
    biases_d = nc.dram_tensor("biases", [128, 33], F32, kind="ExternalInput")
    o_d = nc.dram_tensor("o", [BL, PRED, ENC], F32, kind="ExternalOutput")

    with tile.TileContext(nc) as tc:
        with (
            tc.tile_pool(name="wp", bufs=1) as wp,
            tc.tile_pool(name="hpa", bufs=2) as hpa,
            tc.tile_pool(name="hpb", bufs=2) as hpb,
            tc.tile_pool(name="xsp", bufs=4) as xsp,
            tc.tile_pool(name="embp", bufs=3) as embp,
            tc.tile_pool(name="wk", bufs=1) as wk,
            tc.tile_pool(name="rzp", bufs=2) as rzp,
            tc.tile_pool(name="xbp", bufs=2) as xbp,
            tc.tile_pool(name="xnp", bufs=2) as xnp,
            tc.tile_pool(name="dg", bufs=1) as dg,
            tc.tile_pool(name="dwk", bufs=2) as dwk,
            tc.tile_pool(name="dk1", bufs=1) as dk1,
            tc.tile_pool(name="ytp", bufs=2) as ytp,
            tc.tile_pool(name="hyp", bufs=2) as hyp,
            tc.tile_pool(name="psum", bufs=8, space="PSUM") as pp,
        ):
            halves = {}

            class H:
                pass

            for hname, b0, hp in (("A", 0, hpa), ("B", BH, hpb)):
                hh = H()
                hh.name, hh.b0, hh.hp = hname, b0, hp
                hh.xsf = {}
                halves[hname] = hh

            def load_xs(hh, t):
                xsf = xsp.tile([66, C], F32, tag="xsf")
                hh.xsf[t] = xsf
                nc.sync.dma_start(
                    xsf[0:64, :].rearrange("k (b c) -> k b c", b=BH),
                    x_d[hh.b0:hh.b0 + BH, t * SEG:(t + 1) * SEG, :]
                    .rearrange("b k c -> k b c"))
                nc.sync.dma_start(xsf[64:65, :],
                                  lastrow_d[:, hh.b0 * ENC: hh.b0 * ENC + C])
                nc.sync.dma_start(xsf[65:66, :],
                                  ones_d[:, hh.b0 * ENC: hh.b0 * ENC + C])
                return xsf

            A0h = halves["A"]
            xsfA0 = load_xs(A0h, 0)
            load_xs(A0h, 1)
            load_xs(A0h, 2)
            wemb = wp.tile([66, D], BF16, tag="wemb")
            nc.sync.dma_start(wemb[:], wemb_d[:])
            bia = wp.tile([128, 33], F32, tag="bia")
            nc.sync.dma_start(bia[:], biases_d[:])
            wx8 = wp.tile([128, 8 * 512], F8, tag="wx8")
            nc.sync.dma_start(wx8[:], wx8_d[:])

            def wload(name, dram, width):
                t = wp.tile([128, KC * width], BF16, tag=name)
                nc.sync.dma_start(t[:].rearrange("p (kc j) -> p kc j", kc=KC),
                                  dram[:].rearrange("(kc p) j -> p kc j", p=128))
                return t

            wxn = wload("wxn", wxn_d, D)
            wh8 = wp.tile([128, 8 * 512], F8, tag="wh8")
            nc.sync.dma_start(wh8[:], wh8_d[:])
            whn = wload("whn", whn_d, D)
            wres = wload("wres", wres_d, D)
            whd = wload("whd", whd_d, G3)
            wpred = wload("wpred", wpred_d, SEG)
            gxd = wp.tile([128, MC * SNY * ENC], BF16, tag="gxd")
            nc.sync.dma_start(gxd[:], gxd_d[:])
            gxdt = wp.tile([128, SNY * 6 * 128], BF16, tag="gxdt")
            nc.sync.dma_start(gxdt[:], gxdt_d[:])
            ident = wp.tile([128, 128], BF16, tag="ident")
            nc.sync.dma_start(ident[:], ident_d[:])
            selm = wp.tile([128, C], BF16, tag="selm")
            nc.sync.dma_start(selm[:], selm_d[:])
            last64 = wp.tile([64, R], F32, tag="last64")
            nc.sync.dma_start(last64[:], lastrow_d[:].partition_broadcast(64))

            def wsl(w, kc, mc, width=G3):
                return w[:, kc * width + mc * 128: kc * width + mc * 128 + 128]

            def wdr8(w, j, mc):
                """fp8 DR stationary [128, 2, 128] for rz chunk mc, pair j."""
                s = mc * 512 + j * 256
                return w[:, s:s + 256].rearrange("p (two m) -> p two m", two=2)

            def mdr(tl, j):
                """fp8 DR moving [128, 2, C] for kc-pair j."""
                return tl[:, j * 2 * C:(j + 1) * 2 * C].rearrange(
                    "p (two c) -> p two c", two=2)

            def cast_xs(hh, xsf):
                xsb = xbp.tile([66, C], BF16, tag="xsb")
                nc.scalar.activation(xsb[:], xsf[:], AF.Identity)
                return xsb

            def emb_mms(hh, xsf):
                """emb: K=66 bf16 matmuls (rows 64/65 fold -seq_last and
                b_emb); per-chunk sigmoid (no bias) + silu tt -> embB bf16;
                embT8 fp8 copy on the idle GpSimd for the rz DR matmuls."""
                xsb = cast_xs(hh, xsf)
                embB = embp.tile([128, KC * C], BF16, tag="embB")
                for mc in range(KC):
                    sl = slice(mc * C, (mc + 1) * C)
                    ps = pp.tile([128, C], F32, tag="ps")
                    nc.tensor.matmul(ps[:], wemb[:, mc * 128:(mc + 1) * 128],
                                     xsb[:], start=True, stop=True)
                    nc.scalar.activation(embB[:, sl], ps[:], AF.Sigmoid)
                    nc.vector.tensor_tensor(embB[:, sl], ps[:], embB[:, sl],
                                            ALU.mult)
                embT8 = embp.tile([128, KC * C], F8, tag="embT8")
                nc.vector.tensor_scalar(embT8[:], embB[:], 1.0, None, ALU.mult)
                return (embB, embT8)

            def psxn_mms(hh, embB):
                """x-side n-gate pre-acts (bf16), drained with bias bih_n."""
                xn = xnp.tile([128, KC * C], BF16, tag="xn")
                for mc in range(KC):
                    ps = pp.tile([128, C], F32, tag="ps")
                    for kc in range(KC):
                        nc.tensor.matmul(ps[:], wsl(wxn, kc, mc, D),
                                         embB[:, kc * C:(kc + 1) * C],
                                         start=(kc == 0), stop=(kc == KC - 1))
                    nc.scalar.activation(xn[:, mc * C:(mc + 1) * C], ps[:],
                                         AF.Identity,
                                         bias=bia[:, BC_XN + mc: BC_XN + mc + 1])
                return xn

            def enc_step(hh, t):
                embB, embT8 = hh.embT[t]
                xn, hT = hh.xn, hh.hT
                if t < SNX - 3:
                    load_xs(hh, t + 3)
                # A: rz: x-side fp8 DR (x8) + h-side bf16 (x8)
                hT8 = hh.hT8
                rz = rzp.tile([128, 8 * C], BF16, tag="rz")
                for mc in range(8):
                    ps = pp.tile([128, C], F32, tag="ps")
                    nk = KP if t > 0 else 0
                    for j in range(KP):
                        nc.tensor.matmul(ps[:], wdr8(wx8, j, mc), mdr(embT8, j),
                                         start=(j == 0),
                                         stop=(nk == 0 and j == KP - 1),
                                         perf_mode=DR)
                    for j in range(nk):
                        nc.tensor.matmul(ps[:], wdr8(wh8, j, mc), mdr(hT8, j),
                                         start=False, stop=(j == nk - 1),
                                         perf_mode=DR)
                    nc.scalar.activation(rz[:, mc * C:(mc + 1) * C], ps[:],
                                         AF.Sigmoid, scale=1.0 / XS,
                                         bias=bia[:, BC_RZ + mc: BC_RZ + mc + 1])
                # B: h-side n-gate; per-mc chain t1,t2,tanh,hc
                nsb = wk.tile([128, 2 * C], BF16, tag="nsb")
                hc = wk.tile([128, KC * C], BF16, tag="hc")
                t12 = wk.tile([128, C], BF16, tag="t12")
                for mc in range(4):
                    rsl = rz[:, mc * C:(mc + 1) * C]
                    zsl = rz[:, (4 + mc) * C:(5 + mc) * C]
                    nsl = nsb[:, (mc % 2) * C:(mc % 2 + 1) * C]
                    csl = hc[:, mc * C:(mc + 1) * C]
                    t1 = t12[:, 0:C]
                    if t > 0:
                        ps = pp.tile([128, C], F32, tag="ps")
                        for kc in range(KC):
                            nc.tensor.matmul(ps[:], wsl(whn, kc, mc, D),
                                             hT[:, kc * C:(kc + 1) * C],
                                             start=(kc == 0),
                                             stop=(kc == KC - 1))
                        nc.vector.scalar_tensor_tensor(
                            t1, ps[:], bia[:, BC_HN + mc: BC_HN + mc + 1],
                            rsl, ALU.add, ALU.mult)
                    else:
                        nc.vector.tensor_scalar(
                            t1, rsl, bia[:, BC_HN + mc: BC_HN + mc + 1],
                            None, ALU.mult)
                    nc.vector.tensor_tensor(t1, xn[:, mc * C:(mc + 1) * C],
                                            t1, ALU.add)
                    nc.scalar.activation(nsl, t1, AF.Tanh)
                    if t > 0:
                        nc.vector.tensor_tensor(csl, hT[:, mc * C:(mc + 1) * C],
                                                nsl, ALU.subtract)
                        nc.vector.tensor_tensor(csl, csl, zsl, ALU.mult)
                        nc.vector.tensor_tensor(csl, csl, nsl, ALU.add)
                    else:
                        nc.vector.tensor_tensor(csl, zsl, nsl, ALU.mult)
                        nc.vector.tensor_tensor(csl, nsl, csl, ALU.subtract)
                # C: emb two steps ahead; D: xn for t+1
                if t < SNX - 2:
                    hh.embT[t + 2] = emb_mms(hh, hh.xsf[t + 2])
                if t < SNX - 1:
                    xn_next = psxn_mms(hh, hh.embT[t + 1][0])
                # G: res projection, kc-outer so first MMs need only hc[0]
                psr = [pp.tile([128, C], F32, tag="ps", name=f"res{mc}")
                       for mc in range(KC)]
                for kc in range(KC):
                    for mc in range(KC):
                        nc.tensor.matmul(psr[mc][:], wsl(wres, kc, mc, D),
                                         hc[:, kc * C:(kc + 1) * C],
                                         start=(kc == 0), stop=(kc == KC - 1))
                hT_new = hh.hp.tile([128, KC * C], BF16, tag=f"h{hh.name}",
                                    name=f"h{hh.name}_{t}")
                for mc in range(KC):
                    nc.vector.scalar_tensor_tensor(
                        hT_new[:, mc * C:(mc + 1) * C], psr[mc][:],
                        bia[:, BC_RES + mc: BC_RES + mc + 1],
                        embB[:, mc * C:(mc + 1) * C], ALU.add, ALU.add)
                hT8_new = hh.hp.tile([128, KC * C], F8, tag=f"h8{hh.name}",
                                     name=f"h8{hh.name}_{t}")
                nc.vector.tensor_scalar(hT8_new[:], hT_new[:], 1.0, None,
                                        ALU.mult)
                del hh.embT[t]
                hh.hT = hT_new
                hh.hT8 = hT8_new
                if t < SNX - 1:
                    hh.xn = xn_next

            def ghd_mms(hh, ghd=None, mcs=range(MC)):
                if ghd is None:
                    ghd = dg.tile([128, MC * C], BF16, tag="ghd")
                for mc in mcs:
                    ps = pp.tile([128, C], F32, tag="ps")
                    for kc in range(KC):
                        nc.tensor.matmul(ps[:], wsl(whd, kc, mc),
                                         hh.hT[:, kc * C:(kc + 1) * C],
                                         start=(kc == 0), stop=(kc == KC - 1))
                    nc.scalar.activation(ghd[:, mc * C:(mc + 1) * C], ps[:],
                                         AF.Identity,
                                         bias=bia[:, BC_GBHH + mc: BC_GBHH + mc + 1])
                return ghd

            def gxv(mc, s):
                v = gxd[:, mc * (SNY * ENC) + s * ENC: mc * (SNY * ENC) + (s + 1) * ENC]
                return v.unsqueeze(1).to_broadcast((128, BH, ENC))

            def dec_pred_store(hh, s, hy):
                hy2, nd = hy
                ps = pp.tile([64, C], F32, tag="ps")
                for kc in range(KC):
                    nc.tensor.matmul(ps[:], wpred[:, kc * SEG:(kc + 1) * SEG],
                                     hy2[:, kc * C:(kc + 1) * C],
                                     start=(kc == 0), stop=False)
                for kc in range(KC):
                    nc.tensor.matmul(ps[:], wpred[:, kc * SEG:(kc + 1) * SEG],
                                     nd[:, kc * C:(kc + 1) * C],
                                     start=False, stop=(kc == KC - 1))
                yt = ytp.tile([64, C], F32, tag="yt")
                nc.scalar.activation(yt[:], ps[:], AF.Identity,
                                     bias=bia[0:64, BC_PRED: BC_PRED + 1])
                nc.vector.tensor_tensor(
                    yt[:], yt[:], last64[:, hh.b0 * ENC: hh.b0 * ENC + C], ALU.add)
                nc.sync.dma_start(
                    o_d[hh.b0:hh.b0 + BH, s * SEG:(s + 1) * SEG, :]
                    .rearrange("b k c -> k b c"),
                    yt[:].rearrange("k (b c) -> k b c", b=BH))

            def dec_hy(hh, rzd, nd):
                """pred(hy) = pred(z*(h-n)) + pred(n): final add rides the
                pred psum accumulation instead of a serial DVE op."""
                hy2 = hyp.tile([128, KC * C], BF16, tag="hy")
                nc.vector.tensor_tensor(hy2[:], hh.hT[:], nd[:], ALU.subtract)
                nc.vector.tensor_tensor(hy2[:], hy2[:], rzd[:, 4 * C:8 * C],
                                        ALU.mult)
                return (hy2, nd)

            def gxv4(mc0, nmc, s):
                """gxd 4D view [128, nmc, BH(bcast), ENC] for chunk range."""
                v = gxd[:].rearrange("p (m sc) -> p m sc", m=MC)
                v = v[:, mc0:mc0 + nmc, s * ENC:(s + 1) * ENC]
                return v.unsqueeze(2).to_broadcast((128, nmc, BH, ENC))

            def dec_chunk_dve(hh, ghd, s):
                u = dwk.tile([128, 8 * C], BF16, tag="rzd")
                nc.vector.tensor_tensor(
                    u[:].rearrange("p (m b c) -> p m b c", m=8, b=BH),
                    ghd[:, 0:8 * C].rearrange("p (m b c) -> p m b c", m=8, b=BH),
                    gxv4(0, 8, s), ALU.add)
                rzd = u
                nc.scalar.activation(rzd[:], u[:], AF.Sigmoid)
                t1d = dk1.tile([128, 4 * C], BF16, tag="dt1")
                nc.vector.tensor_tensor(t1d[:], ghd[:, 8 * C:12 * C],
                                        rzd[:, 0:4 * C], ALU.mult)
                t2d = dwk.tile([128, 4 * C], BF16, tag="dt2")
                nc.vector.tensor_tensor(
                    t2d[:].rearrange("p (m b c) -> p m b c", m=4, b=BH),
                    t1d[:].rearrange("p (m b c) -> p m b c", m=4, b=BH),
                    gxv4(8, 4, s), ALU.add)
                nd = t2d
                nc.scalar.activation(nd[:], t2d[:], AF.Tanh)
                return dec_hy(hh, rzd, nd)

            def warm(n):
                for _ in range(n):
                    ps = pp.tile([128, C], F32, tag="ps")
                    nc.tensor.matmul(ps[:], ident[:], selm[:], start=True,
                                     stop=True)

            def sel_part1(hh, ghd, s):
                rzd = dwk.tile([128, 8 * C], BF16, tag="rzd")
                for mc in range(8):
                    ps = pp.tile([128, C], F32, tag="ps")
                    nc.tensor.matmul(ps[:], ident[:], ghd[:, mc * C:(mc + 1) * C],
                                     start=True, stop=False)
                    hi = mc >= 6
                    nc.tensor.matmul(
                        ps[:],
                        gxdt[64 * hi:64 * hi + 64,
                             s * 768 + (mc % 6) * 128: s * 768 + (mc % 6) * 128 + 128],
                        selm[64 * hi:64 * hi + 64, :], start=False, stop=True)
                    nc.scalar.activation(rzd[:, mc * C:(mc + 1) * C], ps[:],
                                         AF.Sigmoid)
                t1d = dk1.tile([128, 4 * C], BF16, tag="dt1")
                nc.vector.tensor_tensor(t1d[:], ghd[:, 8 * C:12 * C],
                                        rzd[:, 0:4 * C], ALU.mult)
                t2d = dwk.tile([128, 4 * C], BF16, tag="dt2")
                for mc in range(4):
                    mca = 8 + mc
                    psn = pp.tile([128, C], F32, tag="ps")
                    nc.tensor.matmul(
                        psn[:],
                        gxdt[64:128,
                             s * 768 + (mca % 6) * 128: s * 768 + (mca % 6) * 128 + 128],
                        selm[64:128, :], start=True, stop=True)
                    nc.vector.tensor_tensor(t2d[:, mc * C:(mc + 1) * C], psn[:],
                                            t1d[:, mc * C:(mc + 1) * C], ALU.add)
                warm(6)
                return (s, rzd, t2d)

            def sel_part2(hh, p1):
                s, rzd, t2d = p1
                nd = t2d
                nc.scalar.activation(nd[:], t2d[:], AF.Tanh)
                warm(4)
                hy = dec_hy(hh, rzd, nd)
                dec_pred_store(hh, s, hy)

            # ================= schedule =================
            A, Bh = halves["A"], halves["B"]
            A.embT = {}
            A.embT[0] = emb_mms(A, xsfA0)
            A.xn = psxn_mms(A, A.embT[0][0])
            A.embT[1] = emb_mms(A, A.xsf[1])
            A.hT = None
            A.hT8 = None
            Bh.embT = {}
            for t in range(SNX):
                enc_step(A, t)
                if t == 12:
                    load_xs(Bh, 0)
                    load_xs(Bh, 1)
                    load_xs(Bh, 2)
                if t == 13:
                    Bh.embT[0] = emb_mms(Bh, Bh.xsf[0])
                if t == 14:
                    Bh.embT[1] = emb_mms(Bh, Bh.xsf[1])
                    Bh.xn = psxn_mms(Bh, Bh.embT[0][0])
            Bh.hT = None
            Bh.hT8 = None
            ghdA = None
            pend = None
            for t in range(SNX):
                enc_step(Bh, t)
                if pend is not None:
                    dec_pred_store(A, pend[0], pend[1])
                    pend = None
                if t == 4:
                    ghdA = ghd_mms(A, mcs=range(0, 3))
                if t in (5, 6, 7):
                    ghd_mms(A, ghd=ghdA, mcs=range(3 * (t - 4), 3 * (t - 3)))
                if t >= 8:
                    pend = (t - 8, dec_chunk_dve(A, ghdA, t - 8))
            warm(10)
            ghdB = ghd_mms(Bh)
            dec_pred_store(A, pend[0], pend[1])
            p1 = sel_part1(Bh, ghdB, 0)
            for s in range(1, SNY):
                p1_next = sel_part1(Bh, ghdB, s)
                sel_part2(Bh, p1)
                p1 = p1_next
            sel_part2(Bh, p1)
    nc.finalize()
    return nc


def _prep_host(inputs):
    f = lambda a: np.ascontiguousarray(a, dtype=np.float32)
    bfc = lambda a: np.ascontiguousarray(a).astype(bf16)

    def blayout(WT):
        """[D, M] -> [128, KC*M] baseline layout (partition=d_in%128)."""
        Dd, M = WT.shape
        return bfc(WT.reshape(KC, 128, M).transpose(1, 0, 2).reshape(128, KC * M))

    W_emb = f(inputs["W_emb"])
    wemb = np.zeros((66, D), np.float32)
    wemb[0:64, :] = W_emb.T
    wemb[64, :] = -W_emb.sum(axis=1)
    wemb[65, :] = f(inputs["b_emb"])
    Wih, Whh = f(inputs["cell_Wih"]), f(inputs["cell_Whh"])
    bih, bhh = f(inputs["cell_bih"]), f(inputs["cell_bhh"])
    resW, resb = f(inputs["res_W"]), f(inputs["res_b"])
    gWih, gWhh = f(inputs["gru_Wih"]), f(inputs["gru_Whh"])
    gbih, gbhh = f(inputs["gru_bih"]), f(inputs["gru_bhh"])
    predW, predb = f(inputs["pred_W"]), f(inputs["pred_b"])
    pos_emb, channel_emb = f(inputs["pos_emb"]), f(inputs["channel_emb"])

    # x-side rz weights: fp8 DR layout, upscaled x8
    WxT = Wih.T                                   # [D, 3D]
    wx8 = np.clip(WxT[:, 0:1024] * XS, -240, 240)  # rz part
    # [d, m] -> [p=d%128, mc*512 + j*256 + k*128 + mcol]
    wx8 = wx8.reshape(KP, 2, 128, 8, 128).transpose(2, 3, 0, 1, 4)
    wx8 = np.ascontiguousarray(wx8.reshape(128, 8 * 512)).astype(f8)
    # x-side n weights: bf16
    wxn = WxT[:, 1024:1536]                       # [D, D]
    # h-side rz weights: fp8 DR layout, upscaled x8; n part bf16 unscaled
    WhT = Whh.T
    wh8 = np.clip(WhT[:, 0:1024] * XS, -240, 240)
    wh8 = wh8.reshape(KP, 2, 128, 8, 128).transpose(2, 3, 0, 1, 4)
    wh8 = np.ascontiguousarray(wh8.reshape(128, 8 * 512)).astype(f8)
    whn = WhT[:, 1024:1536]

    half = D // 2
    pe = np.zeros((D, SNY * ENC), np.float32)
    pe[0:half, :] = np.repeat(pos_emb.T, ENC, axis=1)
    pe[half:, :] = np.tile(channel_emb.T, (1, SNY))
    gx = gWih @ pe + gbih[:, None]
    gxd = np.ascontiguousarray(
        gx.reshape(MC, 128, SNY * ENC).transpose(1, 0, 2).reshape(128, -1))
    gxdt = np.zeros((128, SNY * 6 * 128), np.float32)
    gxg = gx.reshape(MC, 128, SNY, ENC)
    for mc in range(MC):
        rowoff = 64 * (mc // 6)
        for s in range(SNY):
            gxdt[rowoff:rowoff + 64, s * 768 + (mc % 6) * 128:
                 s * 768 + (mc % 6) * 128 + 128] = gxg[mc, :, s, :].T
    ident = np.eye(128, dtype=np.float32)
    selm = np.zeros((128, C), np.float32)
    for c in range(64):
        selm[c, c::64] = 1.0
        selm[64 + c, c::64] = 1.0

    biases = np.zeros((128, 33), np.float32)

    def put(col, vec):
        nch = max(1, len(vec) // 128)
        for i in range(nch):
            seg = vec[i * 128:(i + 1) * 128]
            biases[0:len(seg), col + i] = seg

    put(BC_RZ, (bih + bhh)[0:1024])
    put(BC_HN, bhh[1024:1536])
    put(BC_XN, bih[1024:1536])
    put(BC_RES, resb)
    put(BC_GBHH, gbhh)
    put(BC_PRED, predb)

    return {
        "wemb": bfc(wemb),
        "wx8": wx8,
        "wxn": bfc(wxn),
        "wh8": wh8,
        "whn": bfc(whn),
        "wres": bfc(resW.T),
        "whd": bfc(gWhh.T),
        "wpred": bfc(predW.T),
        "gxd": bfc(gxd), "gxdt": bfc(gxdt),
        "ident": bfc(ident), "selm": bfc(selm),
        "biases": biases,
    }


def kernel(**inputs):
    global _PROGRAM
    if _PROGRAM is None:
        _PROGRAM = _build_program()
    nc = _PROGRAM
    shared = _prep_host(inputs)
    x = np.ascontiguousarray(inputs["x"], dtype=np.float32)
    ones = np.ones((1, R), np.float32)
    in_maps = []
    for c in range(NCORES):
        xs = x[c * BL:(c + 1) * BL]
        m = dict(shared)
        m["x"] = xs
        m["lastrow"] = np.ascontiguousarray(xs[:, -1, :].reshape(1, R))
        m["ones"] = ones
        in_maps.append(m)
    res = run_bass_kernel_spmd(nc, in_maps, list(range(NCORES)))
    out = np.concatenate([res.results[c]["o"] for c in range(NCORES)], axis=0)
    return out.astype(np.float32)


# revision 20
# speedup vs baseline: 1.0368x; 1.0368x over previous
"""Trainium2 Bass kernel for nn_GRUModel (segment-GRU encoder + 1-step GRU decoder).

Sharding: data-parallel over batch B: 8 cores x 16 batches each
(rows n = b_loc*64 + c, R=1024 rows/core). Weights replicated.

v5 mixed precision (validated in numpy at rel_err 1.29e-2 vs 2e-2 gate):
- r/z-gate x-side matmuls: fp8e4 DoubleRow (K=256/instr, halves the
  instruction count; each LDWEIGHTS+MATMUL pair costs ~222ns regardless
  of dtype, so fewer instructions = faster). Weights upscaled x8 to
  clear e4m3's subnormal floor; the 1/8 rides the ACT drain's scale.
  The h-side weights of the same PSUM chains are scaled x8 in bf16
  (exact), so one scale covers the whole pre-activation.
- xn (tanh-sensitive), h-side, res, decoder, pred: bf16.
- emb: f32r matmul (full bf16-speed at N=512) with K=66: row 64 folds
  -seq_last, row 65 (ones) folds b_emb. Kills the f32->bf16 cast and
  the per-chunk biased sigmoid: emb sigmoid is ONE [128,2048] ACT op.
- embB (bf16, from the silu tt) feeds h composition CLEAN of fp8 noise;
  embT8 = fp8 copy (ACT Copy) only feeds the rz DR matmuls.
- n-gate: ACT Tanh directly (sigmoid_and_others table holds
  sigmoid+tanh+identity: zero table swaps).

Elementwise batching: t2/tanh/hc run as [128, 2048] ops (t1 and
hT_new stay per-chunk: their per-partition biases differ per chunk).
PSUM: "ps" [128,512] x4 rotating + "pb" [128,2048] x1 (emb then res
per step, program-ordered).

Schedule skeleton (from the tuned baseline): halves A/B of C=512 rows;
encoder-A then encoder-B with decoder-A interleaved (DVE-broadcast
path), decoder-B in the tail via PE-assisted select-matmul path.
"""
import numpy as np
import ml_dtypes

import concourse.bass as bass
import concourse.bacc as bacc
import concourse.mybir as mybir
from concourse import tile
from concourse.bass_utils import run_bass_kernel_spmd

bf16 = ml_dtypes.bfloat16
f8 = ml_dtypes.float8_e4m3
F32 = mybir.dt.float32
F32R = mybir.dt.float32r
BF16 = mybir.dt.bfloat16
F8 = mybir.dt.float8e4
AF = mybir.ActivationFunctionType
ALU = mybir.AluOpType
DR = mybir.MatmulPerfMode.DoubleRow

B, SEQ, ENC = 128, 1024, 64
D, SEG = 512, 64
SNX = SEQ // SEG          # 16
PRED = 512
SNY = PRED // SEG         # 8
NCORES = 8
BL = B // NCORES          # 16 batches per core
R = BL * ENC              # 1024 rows per core
C = R // 2                # 512 rows per half
BH = BL // 2              # 8 batches per half
KC = D // 128             # 4 contraction chunks
KP = KC // 2              # 2 DoubleRow kc-pairs
G3 = 3 * D
MC = G3 // 128            # 12 gate chunks
XS = 8.0                  # x-side rz upscale

# bias column map
BC_RZ, BC_HN, BC_XN, BC_RES, BC_GBHH, BC_PRED = 0, 8, 12, 16, 20, 32

_PROGRAM = None


def _build_program():
    nc = bacc.Bacc("TRN2", target_bir_lowering=False, debug=False, num_devices=8)
    x_d = nc.dram_tensor("x", [BL, SEQ, ENC], F32, kind="ExternalInput")
    lastrow_d = nc.dram_tensor("lastrow", [1, R], F32, kind="ExternalInput")
    ones_d = nc.dram_tensor("ones", [1, R], F32, kind="ExternalInput")
    wemb_d = nc.dram_tensor("wemb", [66, D], BF16, kind="ExternalInput")
    wx8_d = nc.dram_tensor("wx8", [128, 8 * 512], F8, kind="ExternalInput")
    wxn_d = nc.dram_tensor("wxn", [D, D], BF16, kind="ExternalInput")
    wh8_d = nc.dram_tensor("wh8", [128, 8 * 512], F8, kind="ExternalInput")
    whn_d = nc.dram_tensor("whn", [D, D], BF16, kind="ExternalInput")
    wres_d = nc.dram_tensor("wres", [D, D], BF16, kind="ExternalInput")
    whd_d = nc.dram_tensor("whd", [D, G3], BF16, kind="ExternalInput")
    wpred_d = nc.dram_tensor("wpred", [D, SEG], BF16, kind="ExternalInput")
    gxd_d = nc.dram_tensor("gxd", [128, MC * SNY * ENC], BF16, kind="ExternalInput")
    gxdt_d = nc.dram_tensor("gxdt", [128, SNY * 6 * 128], BF16, kind="ExternalInput")
    ident_d = nc.dram_tensor("ident", [128, 128], BF16, kind="ExternalInput")
    selm_d = nc.dram_tensor("selm", [128, C], BF16, kind="ExternalInput")
    biases_d = nc.dram_tensor("biases", [128, 33], F32, kind="ExternalInput")
    o_d = nc.dram_tensor("o", [BL, PRED, ENC], F32, kind="ExternalOutput")

    with tile.TileContext(nc) as tc:
        with (
            tc.tile_pool(name="wp", bufs=1) as wp,
            tc.tile_pool(name="hpa", bufs=2) as hpa,
            tc.tile_pool(name="hpb", bufs=2) as hpb,
            tc.tile_pool(name="xsp", bufs=4) as xsp,
            tc.tile_pool(name="embp", bufs=3) as embp,
            tc.tile_pool(name="wk", bufs=1) as wk,
            tc.tile_pool(name="xnp", bufs=2) as xnp,
            tc.tile_pool(name="dg", bufs=1) as dg,
            tc.tile_pool(name="dwk", bufs=2) as dwk,
            tc.tile_pool(name="dk1", bufs=1) as dk1,
            tc.tile_pool(name="ytp", bufs=2) as ytp,
            tc.tile_pool(name="hyp", bufs=2) as hyp,
            tc.tile_pool(name="psum", bufs=8, space="PSUM") as pp,
        ):
            halves = {}

            class H:
                pass

            for hname, b0, hp in (("A", 0, hpa), ("B", BH, hpb)):
                hh = H()
                hh.name, hh.b0, hh.hp = hname, b0, hp
                hh.xsf = {}
                halves[hname] = hh

            def load_xs(hh, t):
                xsf = xsp.tile([66, C], F32, tag="xsf")
                hh.xsf[t] = xsf
                nc.sync.dma_start(
                    xsf[0:64, :].rearrange("k (b c) -> k b c", b=BH),
                    x_d[hh.b0:hh.b0 + BH, t * SEG:(t + 1) * SEG, :]
                    .rearrange("b k c -> k b c"))
                nc.sync.dma_start(xsf[64:65, :],
                                  lastrow_d[:, hh.b0 * ENC: hh.b0 * ENC + C])
                nc.sync.dma_start(xsf[65:66, :],
                                  ones_d[:, hh.b0 * ENC: hh.b0 * ENC + C])
                return xsf

            A0h = halves["A"]
            xsfA0 = load_xs(A0h, 0)
            load_xs(A0h, 1)
            load_xs(A0h, 2)
            wemb = wp.tile([66, D], BF16, tag="wemb")
            nc.sync.dma_start(wemb[:], wemb_d[:])
            bia = wp.tile([128, 33], F32, tag="bia")
            nc.sync.dma_start(bia[:], biases_d[:])
            wx8 = wp.tile([128, 8 * 512], F8, tag="wx8")
            nc.sync.dma_start(wx8[:], wx8_d[:])

            def wload(name, dram, width):
                t = wp.tile([128, KC * width], BF16, tag=name)
                nc.sync.dma_start(t[:].rearrange("p (kc j) -> p kc j", kc=KC),
                                  dram[:].rearrange("(kc p) j -> p kc j", p=128))
                return t

            wxn = wload("wxn", wxn_d, D)
            wh8 = wp.tile([128, 8 * 512], F8, tag="wh8")
            nc.sync.dma_start(wh8[:], wh8_d[:])
            whn = wload("whn", whn_d, D)
            wres = wload("wres", wres_d, D)
            whd = wload("whd", whd_d, G3)
            wpred = wload("wpred", wpred_d, SEG)
            gxd = wp.tile([128, MC * SNY * ENC], BF16, tag="gxd")
            nc.sync.dma_start(gxd[:], gxd_d[:])
            gxdt = wp.tile([128, SNY * 6 * 128], BF16, tag="gxdt")
            nc.sync.dma_start(gxdt[:], gxdt_d[:])
            ident = wp.tile([128, 128], BF16, tag="ident")
            nc.sync.dma_start(ident[:], ident_d[:])
            selm = wp.tile([128, C], BF16, tag="selm")
            nc.sync.dma_start(selm[:], selm_d[:])
            last64 = wp.tile([64, R], F32, tag="last64")
            nc.sync.dma_start(last64[:], lastrow_d[:].partition_broadcast(64))

            def wsl(w, kc, mc, width=G3):
                return w[:, kc * width + mc * 128: kc * width + mc * 128 + 128]

            def wdr8(w, j, mc):
                """fp8 DR stationary [128, 2, 128] for rz chunk mc, pair j."""
                s = mc * 512 + j * 256
                return w[:, s:s + 256].rearrange("p (two m) -> p two m", two=2)

            def mdr(tl, j):
                """fp8 DR moving [128, 2, C] for kc-pair j."""
                return tl[:, j * 2 * C:(j + 1) * 2 * C].rearrange(
                    "p (two c) -> p two c", two=2)

            def cast_xs(hh, xsf):
                xsb = xsp.tile([66, C], BF16, tag="xsb")
                nc.scalar.activation(xsb[:], xsf[:], AF.Identity)
                return xsb

            def emb_mms(hh, xsf):
                """emb: K=66 bf16 matmuls (rows 64/65 fold -seq_last and
                b_emb); per-chunk sigmoid (no bias) + silu tt -> embB bf16;
                embT8 fp8 copy on the idle GpSimd for the rz DR matmuls."""
                xsb = cast_xs(hh, xsf)
                embB = embp.tile([128, KC * C], BF16, tag="embB")
                sg = wk.tile([128, KC * C], BF16, tag="sg")
                for mc in range(KC):
                    sl = slice(mc * C, (mc + 1) * C)
                    ps = pp.tile([128, C], F32, tag="ps")
                    nc.tensor.matmul(ps[:], wemb[:, mc * 128:(mc + 1) * 128],
                                     xsb[:], start=True, stop=True)
                    nc.scalar.activation(sg[:, sl], ps[:], AF.Sigmoid)
                    nc.vector.tensor_tensor(embB[:, sl], ps[:], sg[:, sl],
                                            ALU.mult)
                embT8 = embp.tile([128, KC * C], F8, tag="embT8")
                nc.vector.tensor_scalar(embT8[:], embB[:], 1.0, None, ALU.mult)
                return (embB, embT8)

            def psxn_mms(hh, embB):
                """x-side n-gate pre-acts (bf16), drained with bias bih_n."""
                xn = xnp.tile([128, KC * C], BF16, tag="xn")
                for mc in range(KC):
                    ps = pp.tile([128, C], F32, tag="ps")
                    for kc in range(KC):
                        nc.tensor.matmul(ps[:], wsl(wxn, kc, mc, D),
                                         embB[:, kc * C:(kc + 1) * C],
                                         start=(kc == 0), stop=(kc == KC - 1))
                    nc.scalar.activation(xn[:, mc * C:(mc + 1) * C], ps[:],
                                         AF.Identity,
                                         bias=bia[:, BC_XN + mc: BC_XN + mc + 1])
                return xn

            def enc_step(hh, t):
                embB, embT8 = hh.embT[t]
                xn, hT = hh.xn, hh.hT
                if t < SNX - 3:
                    load_xs(hh, t + 3)
                # A: rz: x-side fp8 DR (x8) + h-side bf16 (x8)
                hT8 = hh.hT8
                rz = wk.tile([128, 8 * C], BF16, tag="rz")
                for mc in range(8):
                    ps = pp.tile([128, C], F32, tag="ps")
                    nk = KP if t > 0 else 0
                    for j in range(KP):
                        nc.tensor.matmul(ps[:], wdr8(wx8, j, mc), mdr(embT8, j),
                                         start=(j == 0),
                                         stop=(nk == 0 and j == KP - 1),
                                         perf_mode=DR)
                    for j in range(nk):
                        nc.tensor.matmul(ps[:], wdr8(wh8, j, mc), mdr(hT8, j),
                                         start=False, stop=(j == nk - 1),
                                         perf_mode=DR)
                    nc.scalar.activation(rz[:, mc * C:(mc + 1) * C], ps[:],
                                         AF.Sigmoid, scale=1.0 / XS,
                                         bias=bia[:, BC_RZ + mc: BC_RZ + mc + 1])
                # B: h-side n-gate; per-mc chain t1,t2,tanh,hc
                nsb = wk.tile([128, 4 * C], BF16, tag="nsb")
                hc = wk.tile([128, KC * C], BF16, tag="hc")
                t12 = wk.tile([128, 2 * C], BF16, tag="t12")
                for mc in range(4):
                    rsl = rz[:, mc * C:(mc + 1) * C]
                    zsl = rz[:, (4 + mc) * C:(5 + mc) * C]
                    nsl = nsb[:, mc * C:(mc + 1) * C]
                    csl = hc[:, mc * C:(mc + 1) * C]
                    t1 = t12[:, 0:C]
                    t2 = t12[:, C:2 * C]
                    if t > 0:
                        ps = pp.tile([128, C], F32, tag="ps")
                        for kc in range(KC):
                            nc.tensor.matmul(ps[:], wsl(whn, kc, mc, D),
                                             hT[:, kc * C:(kc + 1) * C],
                                             start=(kc == 0),
                                             stop=(kc == KC - 1))
                        nc.vector.scalar_tensor_tensor(
                            t1, ps[:], bia[:, BC_HN + mc: BC_HN + mc + 1],
                            rsl, ALU.add, ALU.mult)
                    else:
                        nc.vector.tensor_scalar(
                            t1, rsl, bia[:, BC_HN + mc: BC_HN + mc + 1],
                            None, ALU.mult)
                    nc.vector.tensor_tensor(t2, xn[:, mc * C:(mc + 1) * C],
                                            t1, ALU.add)
                    nc.scalar.activation(nsl, t2, AF.Tanh)
                    if t > 0:
                        nc.vector.tensor_tensor(csl, hT[:, mc * C:(mc + 1) * C],
                                                nsl, ALU.subtract)
                        nc.vector.tensor_tensor(csl, csl, zsl, ALU.mult)
                        nc.vector.tensor_tensor(csl, csl, nsl, ALU.add)
                    else:
                        nc.vector.tensor_tensor(csl, zsl, nsl, ALU.mult)
                        nc.vector.tensor_tensor(csl, nsl, csl, ALU.subtract)
                # C: emb two steps ahead; D: xn for t+1
                if t < SNX - 2:
                    hh.embT[t + 2] = emb_mms(hh, hh.xsf[t + 2])
                if t < SNX - 1:
                    xn_next = psxn_mms(hh, hh.embT[t + 1][0])
                # G: res projection, kc-outer so first MMs need only hc[0]
                psr = [pp.tile([128, C], F32, tag="ps", name=f"res{mc}")
                       for mc in range(KC)]
                for kc in range(KC):
                    for mc in range(KC):
                        nc.tensor.matmul(psr[mc][:], wsl(wres, kc, mc, D),
                                         hc[:, kc * C:(kc + 1) * C],
                                         start=(kc == 0), stop=(kc == KC - 1))
                hT_new = hh.hp.tile([128, KC * C], BF16, tag=f"h{hh.name}",
                                    name=f"h{hh.name}_{t}")
                for mc in range(KC):
                    nc.vector.scalar_tensor_tensor(
                        hT_new[:, mc * C:(mc + 1) * C], psr[mc][:],
                        bia[:, BC_RES + mc: BC_RES + mc + 1],
                        embB[:, mc * C:(mc + 1) * C], ALU.add, ALU.add)
                hT8_new = hh.hp.tile([128, KC * C], F8, tag=f"h8{hh.name}",
                                     name=f"h8{hh.name}_{t}")
                nc.vector.tensor_scalar(hT8_new[:], hT_new[:], 1.0, None,
                                        ALU.mult)
                del hh.embT[t]
                hh.hT = hT_new
                hh.hT8 = hT8_new
                if t < SNX - 1:
                    hh.xn = xn_next

            def ghd_mms(hh, ghd=None, mcs=range(MC)):
                if ghd is None:
                    ghd = dg.tile([128, MC * C], BF16, tag="ghd")
                for mc in mcs:
                    ps = pp.tile([128, C], F32, tag="ps")
                    for kc in range(KC):
                        nc.tensor.matmul(ps[:], wsl(whd, kc, mc),
                                         hh.hT[:, kc * C:(kc + 1) * C],
                                         start=(kc == 0), stop=(kc == KC - 1))
                    nc.scalar.activation(ghd[:, mc * C:(mc + 1) * C], ps[:],
                                         AF.Identity,
                                         bias=bia[:, BC_GBHH + mc: BC_GBHH + mc + 1])
                return ghd

            def gxv(mc, s):
                v = gxd[:, mc * (SNY * ENC) + s * ENC: mc * (SNY * ENC) + (s + 1) * ENC]
                return v.unsqueeze(1).to_broadcast((128, BH, ENC))

            def dec_pred_store(hh, s, hy):
                hy2, nd = hy
                ps = pp.tile([64, C], F32, tag="ps")
                for kc in range(KC):
                    nc.tensor.matmul(ps[:], wpred[:, kc * SEG:(kc + 1) * SEG],
                                     hy2[:, kc * C:(kc + 1) * C],
                                     start=(kc == 0), stop=False)
                for kc in range(KC):
                    nc.tensor.matmul(ps[:], wpred[:, kc * SEG:(kc + 1) * SEG],
                                     nd[:, kc * C:(kc + 1) * C],
                                     start=False, stop=(kc == KC - 1))
                yt = ytp.tile([64, C], F32, tag="yt")
                nc.scalar.activation(yt[:], ps[:], AF.Identity,
                                     bias=bia[0:64, BC_PRED: BC_PRED + 1])
                nc.vector.tensor_tensor(
                    yt[:], yt[:], last64[:, hh.b0 * ENC: hh.b0 * ENC + C], ALU.add)
                nc.sync.dma_start(
                    o_d[hh.b0:hh.b0 + BH, s * SEG:(s + 1) * SEG, :]
                    .rearrange("b k c -> k b c"),
                    yt[:].rearrange("k (b c) -> k b c", b=BH))

            def dec_hy(hh, rzd, nd):
                """pred(hy) = pred(z*(h-n)) + pred(n): final add rides the
                pred psum accumulation instead of a serial DVE op."""
                hy2 = hyp.tile([128, KC * C], BF16, tag="hy")
                nc.vector.tensor_tensor(hy2[:], hh.hT[:], nd[:], ALU.subtract)
                nc.vector.tensor_tensor(hy2[:], hy2[:], rzd[:, 4 * C:8 * C],
                                        ALU.mult)
                return (hy2, nd)

            def gxv4(mc0, nmc, s):
                """gxd 4D view [128, nmc, BH(bcast), ENC] for chunk range."""
                v = gxd[:].rearrange("p (m sc) -> p m sc", m=MC)
                v = v[:, mc0:mc0 + nmc, s * ENC:(s + 1) * ENC]
                return v.unsqueeze(2).to_broadcast((128, nmc, BH, ENC))

            def dec_chunk_dve(hh, ghd, s):
                u = dwk.tile([128, 8 * C], BF16, tag="rzd")
                nc.vector.tensor_tensor(
                    u[:].rearrange("p (m b c) -> p m b c", m=8, b=BH),
                    ghd[:, 0:8 * C].rearrange("p (m b c) -> p m b c", m=8, b=BH),
                    gxv4(0, 8, s), ALU.add)
                rzd = u
                nc.scalar.activation(rzd[:], u[:], AF.Sigmoid)
                t1d = dk1.tile([128, 4 * C], BF16, tag="dt1")
                nc.vector.tensor_tensor(t1d[:], ghd[:, 8 * C:12 * C],
                                        rzd[:, 0:4 * C], ALU.mult)
                t2d = dwk.tile([128, 4 * C], BF16, tag="dt2")
                nc.vector.tensor_tensor(
                    t2d[:].rearrange("p (m b c) -> p m b c", m=4, b=BH),
                    t1d[:].rearrange("p (m b c) -> p m b c", m=4, b=BH),
                    gxv4(8, 4, s), ALU.add)
                nd = t2d
                nc.scalar.activation(nd[:], t2d[:], AF.Tanh)
                return dec_hy(hh, rzd, nd)

            def warm(n):
                for _ in range(n):
                    ps = pp.tile([128, C], F32, tag="ps")
                    nc.tensor.matmul(ps[:], ident[:], selm[:], start=True,
                                     stop=True)

            def sel_part1(hh, ghd, s):
                rzd = dwk.tile([128, 8 * C], BF16, tag="rzd")
                for mc in range(8):
                    ps = pp.tile([128, C], F32, tag="ps")
                    nc.tensor.matmul(ps[:], ident[:], ghd[:, mc * C:(mc + 1) * C],
                                     start=True, stop=False)
                    hi = mc >= 6
                    nc.tensor.matmul(
                        ps[:],
                        gxdt[64 * hi:64 * hi + 64,
                             s * 768 + (mc % 6) * 128: s * 768 + (mc % 6) * 128 + 128],
                        selm[64 * hi:64 * hi + 64, :], start=False, stop=True)
                    nc.scalar.activation(rzd[:, mc * C:(mc + 1) * C], ps[:],
                                         AF.Sigmoid)
                t1d = dk1.tile([128, 4 * C], BF16, tag="dt1")
                nc.vector.tensor_tensor(t1d[:], ghd[:, 8 * C:12 * C],
                                        rzd[:, 0:4 * C], ALU.mult)
                t2d = dwk.tile([128, 4 * C], BF16, tag="dt2")
                for mc in range(4):
                    mca = 8 + mc
                    psn = pp.tile([128, C], F32, tag="ps")
                    nc.tensor.matmul(
                        psn[:],
                        gxdt[64:128,
                             s * 768 + (mca % 6) * 128: s * 768 + (mca % 6) * 128 + 128],
                        selm[64:128, :], start=True, stop=True)
                    nc.vector.tensor_tensor(t2d[:, mc * C:(mc + 1) * C], psn[:],
                                            t1d[:, mc * C:(mc + 1) * C], ALU.add)
                warm(6)
                return (s, rzd, t2d)

            def sel_part2(hh, p1):
                s, rzd, t2d = p1
                nd = t2d
                nc.scalar.activation(nd[:], t2d[:], AF.Tanh)
                warm(4)
                hy = dec_hy(hh, rzd, nd)
                dec_pred_store(hh, s, hy)

            # ================= schedule =================
            A, Bh = halves["A"], halves["B"]
            A.embT = {}
            A.embT[0] = emb_mms(A, xsfA0)
            A.xn = psxn_mms(A, A.embT[0][0])
            A.embT[1] = emb_mms(A, A.xsf[1])
            A.hT = None
            A.hT8 = None
            Bh.embT = {}
            for t in range(SNX):
                enc_step(A, t)
                if t == 12:
                    load_xs(Bh, 0)
                    load_xs(Bh, 1)
                    load_xs(Bh, 2)
                if t == 13:
                    Bh.embT[0] = emb_mms(Bh, Bh.xsf[0])
                if t == 14:
                    Bh.embT[1] = emb_mms(Bh, Bh.xsf[1])
                    Bh.xn = psxn_mms(Bh, Bh.embT[0][0])
            Bh.hT = None
            Bh.hT8 = None
            ghdA = None
            pend = None
            for t in range(SNX):
                enc_step(Bh, t)
                if pend is not None:
                    dec_pred_store(A, pend[0], pend[1])
                    pend = None
                if t == 4:
                    ghdA = ghd_mms(A, mcs=range(0, 3))
                if t in (5, 6, 7):
                    ghd_mms(A, ghd=ghdA, mcs=range(3 * (t - 4), 3 * (t - 3)))
                if t >= 8:
                    pend = (t - 8, dec_chunk_dve(A, ghdA, t - 8))
            warm(10)
            ghdB = ghd_mms(Bh)
            dec_pred_store(A, pend[0], pend[1])
            p1 = sel_part1(Bh, ghdB, 0)
            for s in range(1, SNY):
                p1_next = sel_part1(Bh, ghdB, s)
                sel_part2(Bh, p1)
                p1 = p1_next
            sel_part2(Bh, p1)
    nc.finalize()
    return nc


def _prep_host(inputs):
    f = lambda a: np.ascontiguousarray(a, dtype=np.float32)
    bfc = lambda a: np.ascontiguousarray(a).astype(bf16)

    def blayout(WT):
        """[D, M] -> [128, KC*M] baseline layout (partition=d_in%128)."""
        Dd, M = WT.shape
        return bfc(WT.reshape(KC, 128, M).transpose(1, 0, 2).reshape(128, KC * M))

    W_emb = f(inputs["W_emb"])
    wemb = np.zeros((66, D), np.float32)
    wemb[0:64, :] = W_emb.T
    wemb[64, :] = -W_emb.sum(axis=1)
    wemb[65, :] = f(inputs["b_emb"])
    Wih, Whh = f(inputs["cell_Wih"]), f(inputs["cell_Whh"])
    bih, bhh = f(inputs["cell_bih"]), f(inputs["cell_bhh"])
    resW, resb = f(inputs["res_W"]), f(inputs["res_b"])
    gWih, gWhh = f(inputs["gru_Wih"]), f(inputs["gru_Whh"])
    gbih, gbhh = f(inputs["gru_bih"]), f(inputs["gru_bhh"])
    predW, predb = f(inputs["pred_W"]), f(inputs["pred_b"])
    pos_emb, channel_emb = f(inputs["pos_emb"]), f(inputs["channel_emb"])

    # x-side rz weights: fp8 DR layout, upscaled x8
    WxT = Wih.T                                   # [D, 3D]
    wx8 = np.clip(WxT[:, 0:1024] * XS, -240, 240)  # rz part
    # [d, m] -> [p=d%128, mc*512 + j*256 + k*128 + mcol]
    wx8 = wx8.reshape(KP, 2, 128, 8, 128).transpose(2, 3, 0, 1, 4)
    wx8 = np.ascontiguousarray(wx8.reshape(128, 8 * 512)).astype(f8)
    # x-side n weights: bf16
    wxn = WxT[:, 1024:1536]                       # [D, D]
    # h-side rz weights: fp8 DR layout, upscaled x8; n part bf16 unscaled
    WhT = Whh.T
    wh8 = np.clip(WhT[:, 0:1024] * XS, -240, 240)
    wh8 = wh8.reshape(KP, 2, 128, 8, 128).transpose(2, 3, 0, 1, 4)
    wh8 = np.ascontiguousarray(wh8.reshape(128, 8 * 512)).astype(f8)
    whn = WhT[:, 1024:1536]

    half = D // 2
    pe = np.zeros((D, SNY * ENC), np.float32)
    pe[0:half, :] = np.repeat(pos_emb.T, ENC, axis=1)
    pe[half:, :] = np.tile(channel_emb.T, (1, SNY))
    gx = gWih @ pe + gbih[:, None]
    gxd = np.ascontiguousarray(
        gx.reshape(MC, 128, SNY * ENC).transpose(1, 0, 2).reshape(128, -1))
    gxdt = np.zeros((128, SNY * 6 * 128), np.float32)
    gxg = gx.reshape(MC, 128, SNY, ENC)
    for mc in range(MC):
        rowoff = 64 * (mc // 6)
        for s in range(SNY):
            gxdt[rowoff:rowoff + 64, s * 768 + (mc % 6) * 128:
                 s * 768 + (mc % 6) * 128 + 128] = gxg[mc, :, s, :].T
    ident = np.eye(128, dtype=np.float32)
    selm = np.zeros((128, C), np.float32)
    for c in range(64):
        selm[c, c::64] = 1.0
        selm[64 + c, c::64] = 1.0

    biases = np.zeros((128, 33), np.float32)

    def put(col, vec):
        nch = max(1, len(vec) // 128)
        for i in range(nch):
            seg = vec[i * 128:(i + 1) * 128]
            biases[0:len(seg), col + i] = seg

    put(BC_RZ, (bih + bhh)[0:1024])
    put(BC_HN, bhh[1024:1536])
    put(BC_XN, bih[1024:1536])
    put(BC_RES, resb)
    put(BC_GBHH, gbhh)
    put(BC_PRED, predb)

    return {
        "wemb": bfc(wemb),
        "wx8": wx8,
        "wxn": bfc(wxn),
        "wh8": wh8,
        "whn": bfc(whn),
        "wres": bfc(resW.T),
        "whd": bfc(gWhh.T),
        "wpred": bfc(predW.T),
        "gxd": bfc(gxd), "gxdt": bfc(gxdt),
        "ident": bfc(ident), "selm": bfc(selm),
        "biases": biases,
    }


def kernel(**inputs):
    global _PROGRAM
    if _PROGRAM is None:
        _PROGRAM = _build_program()
    nc = _PROGRAM
    shared = _prep_host(inputs)
    x = np.ascontiguousarray(inputs["x"], dtype=np.float32)
    ones = np.ones((1, R), np.float32)
    in_maps = []
    for c in range(NCORES):
        xs = x[c * BL:(c + 1) * BL]
        m = dict(shared)
        m["x"] = xs
        m["lastrow"] = np.ascontiguousarray(xs[:, -1, :].reshape(1, R))
        m["ones"] = ones
        in_maps.append(m)
    res = run_bass_kernel_spmd(nc, in_maps, list(range(NCORES)))
    out = np.concatenate([res.results[c]["o"] for c in range(NCORES)], axis=0)
    return out.astype(np.float32)


# revision 22
# speedup vs baseline: 1.0587x; 1.0211x over previous
"""Trainium2 Bass kernel for nn_GRUModel (segment-GRU encoder + 1-step GRU decoder).

Sharding: data-parallel over batch B: 8 cores x 16 batches each
(rows n = b_loc*64 + c, R=1024 rows/core). Weights replicated.

v5 mixed precision (validated in numpy at rel_err 1.29e-2 vs 2e-2 gate):
- r/z-gate x-side matmuls: fp8e4 DoubleRow (K=256/instr, halves the
  instruction count; each LDWEIGHTS+MATMUL pair costs ~222ns regardless
  of dtype, so fewer instructions = faster). Weights upscaled x8 to
  clear e4m3's subnormal floor; the 1/8 rides the ACT drain's scale.
  The h-side weights of the same PSUM chains are scaled x8 in bf16
  (exact), so one scale covers the whole pre-activation.
- xn (tanh-sensitive), h-side, res, decoder, pred: bf16.
- emb: f32r matmul (full bf16-speed at N=512) with K=66: row 64 folds
  -seq_last, row 65 (ones) folds b_emb. Kills the f32->bf16 cast and
  the per-chunk biased sigmoid: emb sigmoid is ONE [128,2048] ACT op.
- embB (bf16, from the silu tt) feeds h composition CLEAN of fp8 noise;
  embT8 = fp8 copy (ACT Copy) only feeds the rz DR matmuls.
- n-gate: ACT Tanh directly (sigmoid_and_others table holds
  sigmoid+tanh+identity: zero table swaps).

Elementwise batching: t2/tanh/hc run as [128, 2048] ops (t1 and
hT_new stay per-chunk: their per-partition biases differ per chunk).
PSUM: "ps" [128,512] x4 rotating + "pb" [128,2048] x1 (emb then res
per step, program-ordered).

Schedule skeleton (from the tuned baseline): halves A/B of C=512 rows;
encoder-A then encoder-B with decoder-A interleaved (DVE-broadcast
path), decoder-B in the tail via PE-assisted select-matmul path.
"""
import numpy as np
import ml_dtypes

import concourse.bass as bass
import concourse.bacc as bacc
import concourse.mybir as mybir
from concourse import tile
from concourse.bass_utils import run_bass_kernel_spmd

bf16 = ml_dtypes.bfloat16
f8 = ml_dtypes.float8_e4m3
F32 = mybir.dt.float32
F32R = mybir.dt.float32r
BF16 = mybir.dt.bfloat16
F8 = mybir.dt.float8e4
AF = mybir.ActivationFunctionType
ALU = mybir.AluOpType
DR = mybir.MatmulPerfMode.DoubleRow

B, SEQ, ENC = 128, 1024, 64
D, SEG = 512, 64
SNX = SEQ // SEG          # 16
PRED = 512
SNY = PRED // SEG         # 8
NCORES = 8
BL = B // NCORES          # 16 batches per core
R = BL * ENC              # 1024 rows per core
C = R // 2                # 512 rows per half
BH = BL // 2              # 8 batches per half
KC = D // 128             # 4 contraction chunks
KP = KC // 2              # 2 DoubleRow kc-pairs
G3 = 3 * D
MC = G3 // 128            # 12 gate chunks
XS = 8.0                  # x-side rz upscale

# bias column map
BC_RZ, BC_HN, BC_XN, BC_RES, BC_GBHH, BC_PRED = 0, 8, 12, 16, 20, 32

_PROGRAM = None


def _build_program():
    nc = bacc.Bacc("TRN2", target_bir_lowering=False, debug=False, num_devices=8)
    x_d = nc.dram_tensor("x", [BL, SEQ, ENC], F32, kind="ExternalInput")
    lastrow_d = nc.dram_tensor("lastrow", [1, R], F32, kind="ExternalInput")
    ones_d = nc.dram_tensor("ones", [1, R], F32, kind="ExternalInput")
    wemb_d = nc.dram_tensor("wemb", [66, D], BF16, kind="ExternalInput")
    wx8_d = nc.dram_tensor("wx8", [128, 8 * 512], F8, kind="ExternalInput")
    wxn_d = nc.dram_tensor("wxn", [D, D], BF16, kind="ExternalInput")
    wh8_d = nc.dram_tensor("wh8", [128, 8 * 512], F8, kind="ExternalInput")
    whn_d = nc.dram_tensor("whn", [D, D], BF16, kind="ExternalInput")
    wres_d = nc.dram_tensor("wres", [D, D], BF16, kind="ExternalInput")
    whd_d = nc.dram_tensor("whd", [D, G3], BF16, kind="ExternalInput")
    wpred_d = nc.dram_tensor("wpred", [D, SEG], BF16, kind="ExternalInput")
    gxd_d = nc.dram_tensor("gxd", [128, MC * SNY * ENC], BF16, kind="ExternalInput")
    gxdt_d = nc.dram_tensor("gxdt", [128, SNY * 6 * 128], BF16, kind="ExternalInput")
    ident_d = nc.dram_tensor("ident", [128, 128], BF16, kind="ExternalInput")
    selm_d = nc.dram_tensor("selm", [128, C], BF16, kind="ExternalInput")
    biases_d = nc.dram_tensor("biases", [128, 33], F32, kind="ExternalInput")
    o_d = nc.dram_tensor("o", [BL, PRED, ENC], F32, kind="ExternalOutput")

    with tile.TileContext(nc) as tc:
        with (
            tc.tile_pool(name="wp", bufs=1) as wp,
            tc.tile_pool(name="hpa", bufs=2) as hpa,
            tc.tile_pool(name="hpb", bufs=2) as hpb,
            tc.tile_pool(name="xsp", bufs=4) as xsp,
            tc.tile_pool(name="embp", bufs=3) as embp,
            tc.tile_pool(name="wk", bufs=1) as wk,
            tc.tile_pool(name="xnp", bufs=2) as xnp,
            tc.tile_pool(name="dg", bufs=1) as dg,
            tc.tile_pool(name="dwk", bufs=2) as dwk,
            tc.tile_pool(name="dk1", bufs=1) as dk1,
            tc.tile_pool(name="ytp", bufs=2) as ytp,
            tc.tile_pool(name="hyp", bufs=2) as hyp,
            tc.tile_pool(name="psum", bufs=8, space="PSUM") as pp,
        ):
            halves = {}

            class H:
                pass

            for hname, b0, hp in (("A", 0, hpa), ("B", BH, hpb)):
                hh = H()
                hh.name, hh.b0, hh.hp = hname, b0, hp
                hh.xsf = {}
                halves[hname] = hh

            def load_xs(hh, t):
                xsf = xsp.tile([66, C], F32, tag="xsf")
                hh.xsf[t] = xsf
                nc.sync.dma_start(
                    xsf[0:64, :].rearrange("k (b c) -> k b c", b=BH),
                    x_d[hh.b0:hh.b0 + BH, t * SEG:(t + 1) * SEG, :]
                    .rearrange("b k c -> k b c"))
                nc.sync.dma_start(xsf[64:65, :],
                                  lastrow_d[:, hh.b0 * ENC: hh.b0 * ENC + C])
                nc.sync.dma_start(xsf[65:66, :],
                                  ones_d[:, hh.b0 * ENC: hh.b0 * ENC + C])
                return xsf

            A0h = halves["A"]
            xsfA0 = load_xs(A0h, 0)
            load_xs(A0h, 1)
            load_xs(A0h, 2)
            wemb = wp.tile([66, D], BF16, tag="wemb")
            nc.sync.dma_start(wemb[:], wemb_d[:])
            bia = wp.tile([128, 33], F32, tag="bia")
            nc.sync.dma_start(bia[:], biases_d[:])
            wx8 = wp.tile([128, 8 * 512], F8, tag="wx8")
            nc.sync.dma_start(wx8[:], wx8_d[:])

            def wload(name, dram, width):
                t = wp.tile([128, KC * width], BF16, tag=name)
                nc.sync.dma_start(t[:].rearrange("p (kc j) -> p kc j", kc=KC),
                                  dram[:].rearrange("(kc p) j -> p kc j", p=128))
                return t

            wxn = wload("wxn", wxn_d, D)
            wh8 = wp.tile([128, 8 * 512], F8, tag="wh8")
            nc.sync.dma_start(wh8[:], wh8_d[:])
            whn = wload("whn", whn_d, D)
            wres = wload("wres", wres_d, D)
            whd = wload("whd", whd_d, G3)
            wpred = wload("wpred", wpred_d, SEG)
            gxd = wp.tile([128, MC * SNY * ENC], BF16, tag="gxd")
            nc.sync.dma_start(gxd[:], gxd_d[:])
            gxdt = wp.tile([128, SNY * 6 * 128], BF16, tag="gxdt")
            nc.sync.dma_start(gxdt[:], gxdt_d[:])
            ident = wp.tile([128, 128], BF16, tag="ident")
            nc.sync.dma_start(ident[:], ident_d[:])
            selm = wp.tile([128, C], BF16, tag="selm")
            nc.sync.dma_start(selm[:], selm_d[:])
            last64 = wp.tile([64, R], F32, tag="last64")
            nc.sync.dma_start(last64[:], lastrow_d[:].partition_broadcast(64))

            def wsl(w, kc, mc, width=G3):
                return w[:, kc * width + mc * 128: kc * width + mc * 128 + 128]

            def wdr8(w, j, mc):
                """fp8 DR stationary [128, 2, 128] for rz chunk mc, pair j."""
                s = mc * 512 + j * 256
                return w[:, s:s + 256].rearrange("p (two m) -> p two m", two=2)

            def mdr(tl, j):
                """fp8 DR moving [128, 2, C] for kc-pair j."""
                return tl[:, j * 2 * C:(j + 1) * 2 * C].rearrange(
                    "p (two c) -> p two c", two=2)

            def cast_xs(hh, xsf):
                xsb = xsp.tile([66, C], BF16, tag="xsb")
                nc.scalar.activation(xsb[:], xsf[:], AF.Identity)
                return xsb

            def emb_mms(hh, xsf):
                """emb: K=66 bf16 matmuls (rows 64/65 fold -seq_last and
                b_emb); per-chunk sigmoid (no bias) + silu tt -> embB bf16;
                embT8 fp8 copy on the idle GpSimd for the rz DR matmuls."""
                xsb = cast_xs(hh, xsf)
                embB = embp.tile([128, KC * C], BF16, tag="embB")
                sg = wk.tile([128, KC * C], BF16, tag="sg")
                for mc in range(KC):
                    sl = slice(mc * C, (mc + 1) * C)
                    ps = pp.tile([128, C], F32, tag="ps")
                    nc.tensor.matmul(ps[:], wemb[:, mc * 128:(mc + 1) * 128],
                                     xsb[:], start=True, stop=True)
                    nc.scalar.activation(sg[:, sl], ps[:], AF.Sigmoid)
                    nc.vector.tensor_tensor(embB[:, sl], ps[:], sg[:, sl],
                                            ALU.mult)
                embT8 = embp.tile([128, KC * C], F8, tag="embT8")
                nc.vector.tensor_scalar(embT8[:], embB[:], 1.0, None, ALU.mult)
                return (embB, embT8)

            def psxn_mms(hh, embB):
                """x-side n-gate pre-acts (bf16), drained with bias bih_n."""
                xn = xnp.tile([128, KC * C], BF16, tag="xn")
                for mc in range(KC):
                    ps = pp.tile([128, C], F32, tag="ps")
                    for kc in range(KC):
                        nc.tensor.matmul(ps[:], wsl(wxn, kc, mc, D),
                                         embB[:, kc * C:(kc + 1) * C],
                                         start=(kc == 0), stop=(kc == KC - 1))
                    nc.scalar.activation(xn[:, mc * C:(mc + 1) * C], ps[:],
                                         AF.Identity,
                                         bias=bia[:, BC_XN + mc: BC_XN + mc + 1])
                return xn

            def enc_step(hh, t):
                embB, embT8 = hh.embT[t]
                xn, hT = hh.xn, hh.hT
                if t < SNX - 3:
                    load_xs(hh, t + 3)
                # A: rz: x-side fp8 DR (x8) + h-side bf16 (x8)
                hT8 = hh.hT8
                rz = wk.tile([128, 8 * C], BF16, tag="rz")
                for mc in range(8):
                    ps = pp.tile([128, C], F32, tag="ps")
                    nk = KP if t > 0 else 0
                    for j in range(KP):
                        nc.tensor.matmul(ps[:], wdr8(wx8, j, mc), mdr(embT8, j),
                                         start=(j == 0),
                                         stop=(nk == 0 and j == KP - 1),
                                         perf_mode=DR)
                    for j in range(nk):
                        nc.tensor.matmul(ps[:], wdr8(wh8, j, mc), mdr(hT8, j),
                                         start=False, stop=(j == nk - 1),
                                         perf_mode=DR)
                    nc.scalar.activation(rz[:, mc * C:(mc + 1) * C], ps[:],
                                         AF.Sigmoid, scale=1.0 / XS,
                                         bias=bia[:, BC_RZ + mc: BC_RZ + mc + 1])
                # B: h-side n-gate; per-mc chain t1,t2,tanh,hc
                nsb = wk.tile([128, 4 * C], BF16, tag="nsb")
                hc = wk.tile([128, KC * C], BF16, tag="hc")
                t12 = wk.tile([128, 2 * C], BF16, tag="t12")
                for mc in range(4):
                    rsl = rz[:, mc * C:(mc + 1) * C]
                    zsl = rz[:, (4 + mc) * C:(5 + mc) * C]
                    nsl = nsb[:, mc * C:(mc + 1) * C]
                    csl = hc[:, mc * C:(mc + 1) * C]
                    t1 = t12[:, 0:C]
                    t2 = t12[:, C:2 * C]
                    if t > 0:
                        ps = pp.tile([128, C], F32, tag="ps")
                        for kc in range(KC):
                            nc.tensor.matmul(ps[:], wsl(whn, kc, mc, D),
                                             hT[:, kc * C:(kc + 1) * C],
                                             start=(kc == 0),
                                             stop=(kc == KC - 1))
                        nc.vector.scalar_tensor_tensor(
                            t1, ps[:], bia[:, BC_HN + mc: BC_HN + mc + 1],
                            rsl, ALU.add, ALU.mult)
                    else:
                        nc.vector.tensor_scalar(
                            t1, rsl, bia[:, BC_HN + mc: BC_HN + mc + 1],
                            None, ALU.mult)
                    nc.vector.tensor_tensor(t2, xn[:, mc * C:(mc + 1) * C],
                                            t1, ALU.add)
                    nc.scalar.activation(nsl, t2, AF.Tanh)
                    if t > 0:
                        nc.vector.tensor_tensor(csl, hT[:, mc * C:(mc + 1) * C],
                                                nsl, ALU.subtract)
                        nc.vector.tensor_tensor(csl, csl, zsl, ALU.mult)
                        nc.vector.tensor_tensor(csl, csl, nsl, ALU.add)
                    else:
                        nc.vector.tensor_tensor(csl, zsl, nsl, ALU.mult)
                        nc.vector.tensor_tensor(csl, nsl, csl, ALU.subtract)
                # C: emb two steps ahead; D: xn for t+1
                if t < SNX - 2:
                    hh.embT[t + 2] = emb_mms(hh, hh.xsf[t + 2])
                if t < SNX - 1:
                    xn_next = psxn_mms(hh, hh.embT[t + 1][0])
                # G: res projection, kc-outer so first MMs need only hc[0]
                psr = [pp.tile([128, C], F32, tag="ps", name=f"res{mc}")
                       for mc in range(KC)]
                for kc in range(KC):
                    for mc in range(KC):
                        nc.tensor.matmul(psr[mc][:], wsl(wres, kc, mc, D),
                                         hc[:, kc * C:(kc + 1) * C],
                                         start=(kc == 0), stop=(kc == KC - 1))
                hT_new = hh.hp.tile([128, KC * C], BF16, tag=f"h{hh.name}",
                                    name=f"h{hh.name}_{t}")
                for mc in range(KC):
                    nc.vector.scalar_tensor_tensor(
                        hT_new[:, mc * C:(mc + 1) * C], psr[mc][:],
                        bia[:, BC_RES + mc: BC_RES + mc + 1],
                        embB[:, mc * C:(mc + 1) * C], ALU.add, ALU.add)
                hT8_new = hh.hp.tile([128, KC * C], F8, tag=f"h8{hh.name}",
                                     name=f"h8{hh.name}_{t}")
                nc.vector.tensor_scalar(hT8_new[:], hT_new[:], 1.0, None,
                                        ALU.mult)
                del hh.embT[t]
                hh.hT = hT_new
                hh.hT8 = hT8_new
                if t < SNX - 1:
                    hh.xn = xn_next

            def ghd_mms(hh, ghd=None, mcs=range(MC)):
                if ghd is None:
                    ghd = dg.tile([128, MC * C], BF16, tag="ghd")
                for mc in mcs:
                    ps = pp.tile([128, C], F32, tag="ps")
                    for kc in range(KC):
                        nc.tensor.matmul(ps[:], wsl(whd, kc, mc),
                                         hh.hT[:, kc * C:(kc + 1) * C],
                                         start=(kc == 0), stop=(kc == KC - 1))
                    nc.scalar.activation(ghd[:, mc * C:(mc + 1) * C], ps[:],
                                         AF.Identity,
                                         bias=bia[:, BC_GBHH + mc: BC_GBHH + mc + 1])
                return ghd

            def gxv(mc, s):
                v = gxd[:, mc * (SNY * ENC) + s * ENC: mc * (SNY * ENC) + (s + 1) * ENC]
                return v.unsqueeze(1).to_broadcast((128, BH, ENC))

            def dec_pred_store(hh, s, hy):
                hy2, nd = hy
                ps = pp.tile([64, C], F32, tag="ps")
                for kc in range(KC):
                    nc.tensor.matmul(ps[:], wpred[:, kc * SEG:(kc + 1) * SEG],
                                     hy2[:, kc * C:(kc + 1) * C],
                                     start=(kc == 0), stop=False)
                for kc in range(KC):
                    nc.tensor.matmul(ps[:], wpred[:, kc * SEG:(kc + 1) * SEG],
                                     nd[:, kc * C:(kc + 1) * C],
                                     start=False, stop=(kc == KC - 1))
                yt = ytp.tile([64, C], F32, tag="yt")
                nc.scalar.activation(yt[:], ps[:], AF.Identity,
                                     bias=bia[0:64, BC_PRED: BC_PRED + 1])
                nc.vector.tensor_tensor(
                    yt[:], yt[:], last64[:, hh.b0 * ENC: hh.b0 * ENC + C], ALU.add)
                nc.sync.dma_start(
                    o_d[hh.b0:hh.b0 + BH, s * SEG:(s + 1) * SEG, :]
                    .rearrange("b k c -> k b c"),
                    yt[:].rearrange("k (b c) -> k b c", b=BH))

            def dec_hy(hh, rzd, nd):
                """pred(hy) = pred(z*(h-n)) + pred(n): final add rides the
                pred psum accumulation instead of a serial DVE op."""
                hy2 = hyp.tile([128, KC * C], BF16, tag="hy")
                nc.vector.tensor_tensor(hy2[:], hh.hT[:], nd[:], ALU.subtract)
                nc.vector.tensor_tensor(hy2[:], hy2[:], rzd[:, 4 * C:8 * C],
                                        ALU.mult)
                return (hy2, nd)

            def gxv4(mc0, nmc, s):
                """gxd 4D view [128, nmc, BH(bcast), ENC] for chunk range."""
                v = gxd[:].rearrange("p (m sc) -> p m sc", m=MC)
                v = v[:, mc0:mc0 + nmc, s * ENC:(s + 1) * ENC]
                return v.unsqueeze(2).to_broadcast((128, nmc, BH, ENC))

            def dec_chunk_dve(hh, ghd, s):
                u = dwk.tile([128, 8 * C], BF16, tag="rzd")
                nc.vector.tensor_tensor(
                    u[:].rearrange("p (m b c) -> p m b c", m=8, b=BH),
                    ghd[:, 0:8 * C].rearrange("p (m b c) -> p m b c", m=8, b=BH),
                    gxv4(0, 8, s), ALU.add)
                rzd = u
                nc.scalar.activation(rzd[:], u[:], AF.Sigmoid)
                t1d = dk1.tile([128, 4 * C], BF16, tag="dt1")
                nc.vector.tensor_tensor(t1d[:], ghd[:, 8 * C:12 * C],
                                        rzd[:, 0:4 * C], ALU.mult)
                t2d = dwk.tile([128, 4 * C], BF16, tag="dt2")
                nc.vector.tensor_tensor(
                    t2d[:].rearrange("p (m b c) -> p m b c", m=4, b=BH),
                    t1d[:].rearrange("p (m b c) -> p m b c", m=4, b=BH),
                    gxv4(8, 4, s), ALU.add)
                nd = t2d
                nc.scalar.activation(nd[:], t2d[:], AF.Tanh)
                return dec_hy(hh, rzd, nd)

            def warm(n):
                for _ in range(n):
                    ps = pp.tile([128, C], F32, tag="ps")
                    nc.tensor.matmul(ps[:], ident[:], selm[:], start=True,
                                     stop=True)

            def sel_part1(hh, ghd, s):
                rzd = dwk.tile([128, 8 * C], BF16, tag="rzd")
                for mc in range(8):
                    ps = pp.tile([128, C], F32, tag="ps")
                    nc.tensor.matmul(ps[:], ident[:], ghd[:, mc * C:(mc + 1) * C],
                                     start=True, stop=False)
                    hi = mc >= 6
                    nc.tensor.matmul(
                        ps[:],
                        gxdt[64 * hi:64 * hi + 64,
                             s * 768 + (mc % 6) * 128: s * 768 + (mc % 6) * 128 + 128],
                        selm[64 * hi:64 * hi + 64, :], start=False, stop=True)
                    nc.scalar.activation(rzd[:, mc * C:(mc + 1) * C], ps[:],
                                         AF.Sigmoid)
                t1d = dk1.tile([128, 4 * C], BF16, tag="dt1")
                nc.vector.tensor_tensor(t1d[:], ghd[:, 8 * C:12 * C],
                                        rzd[:, 0:4 * C], ALU.mult)
                t2d = dwk.tile([128, 4 * C], BF16, tag="dt2")
                for mc in range(4):
                    mca = 8 + mc
                    psn = pp.tile([128, C], F32, tag="ps")
                    nc.tensor.matmul(
                        psn[:],
                        gxdt[64:128,
                             s * 768 + (mca % 6) * 128: s * 768 + (mca % 6) * 128 + 128],
                        selm[64:128, :], start=True, stop=True)
                    nc.vector.tensor_tensor(t2d[:, mc * C:(mc + 1) * C], psn[:],
                                            t1d[:, mc * C:(mc + 1) * C], ALU.add)
                warm(6)
                return (s, rzd, t2d)

            def sel_part2(hh, p1):
                s, rzd, t2d = p1
                nd = t2d
                nc.scalar.activation(nd[:], t2d[:], AF.Tanh)
                warm(4)
                hy = dec_hy(hh, rzd, nd)
                dec_pred_store(hh, s, hy)

            # ================= schedule =================
            A, Bh = halves["A"], halves["B"]
            A.embT = {}
            A.embT[0] = emb_mms(A, xsfA0)
            A.xn = psxn_mms(A, A.embT[0][0])
            A.embT[1] = emb_mms(A, A.xsf[1])
            A.hT = None
            A.hT8 = None
            Bh.embT = {}
            for t in range(SNX):
                enc_step(A, t)
                if t == 12:
                    load_xs(Bh, 0)
                    load_xs(Bh, 1)
                    load_xs(Bh, 2)
                if t == 13:
                    Bh.embT[0] = emb_mms(Bh, Bh.xsf[0])
                if t == 14:
                    Bh.embT[1] = emb_mms(Bh, Bh.xsf[1])
                    Bh.xn = psxn_mms(Bh, Bh.embT[0][0])
            Bh.hT = None
            Bh.hT8 = None
            ghdA = None
            pend = None
            for t in range(SNX):
                enc_step(Bh, t)
                if pend is not None:
                    dec_pred_store(A, pend[0], pend[1])
                    pend = None
                if t == 4:
                    ghdA = ghd_mms(A, mcs=range(0, 3))
                if t in (5, 6, 7):
                    ghd_mms(A, ghd=ghdA, mcs=range(3 * (t - 4), 3 * (t - 3)))
                if t >= 8:
                    pend = (t - 8, dec_chunk_dve(A, ghdA, t - 8))
            warm(10)
            ghdB = ghd_mms(Bh)
            dec_pred_store(A, pend[0], pend[1])
            p1 = sel_part1(Bh, ghdB, 0)
            for s in range(1, SNY):
                p1_next = sel_part1(Bh, ghdB, s)
                sel_part2(Bh, p1)
                p1 = p1_next
            sel_part2(Bh, p1)
    nc.finalize()
    return nc


def _prep_host(inputs):
    f = lambda a: np.ascontiguousarray(a, dtype=np.float32)
    bfc = lambda a: np.ascontiguousarray(a).astype(bf16)

    def blayout(WT):
        """[D, M] -> [128, KC*M] baseline layout (partition=d_in%128)."""
        Dd, M = WT.shape
        return bfc(WT.reshape(KC, 128, M).transpose(1, 0, 2).reshape(128, KC * M))

    W_emb = f(inputs["W_emb"])
    wemb = np.zeros((66, D), np.float32)
    wemb[0:64, :] = W_emb.T
    wemb[64, :] = -W_emb.sum(axis=1)
    wemb[65, :] = f(inputs["b_emb"])
    Wih, Whh = f(inputs["cell_Wih"]), f(inputs["cell_Whh"])
    bih, bhh = f(inputs["cell_bih"]), f(inputs["cell_bhh"])
    resW, resb = f(inputs["res_W"]), f(inputs["res_b"])
    gWih, gWhh = f(inputs["gru_Wih"]), f(inputs["gru_Whh"])
    gbih, gbhh = f(inputs["gru_bih"]), f(inputs["gru_bhh"])
    predW, predb = f(inputs["pred_W"]), f(inputs["pred_b"])
    pos_emb, channel_emb = f(inputs["pos_emb"]), f(inputs["channel_emb"])

    # x-side rz weights: fp8 DR layout, upscaled x8
    WxT = Wih.T                                   # [D, 3D]
    wx8 = np.clip(WxT[:, 0:1024] * XS, -240, 240)  # rz part
    # [d, m] -> [p=d%128, mc*512 + j*256 + k*128 + mcol]
    wx8 = wx8.reshape(KP, 2, 128, 8, 128).transpose(2, 3, 0, 1, 4)
    wx8 = np.ascontiguousarray(wx8.reshape(128, 8 * 512)).astype(f8)
    # x-side n weights: bf16
    wxn = WxT[:, 1024:1536]                       # [D, D]
    # h-side rz weights: fp8 DR layout, upscaled x8; n part bf16 unscaled
    WhT = Whh.T
    wh8 = np.clip(WhT[:, 0:1024] * XS, -240, 240)
    wh8 = wh8.reshape(KP, 2, 128, 8, 128).transpose(2, 3, 0, 1, 4)
    wh8 = np.ascontiguousarray(wh8.reshape(128, 8 * 512)).astype(f8)
    whn = WhT[:, 1024:1536]

    half = D // 2
    pe = np.zeros((D, SNY * ENC), np.float32)
    pe[0:half, :] = np.repeat(pos_emb.T, ENC, axis=1)
    pe[half:, :] = np.tile(channel_emb.T, (1, SNY))
    gx = gWih @ pe + gbih[:, None]
    gxd = np.ascontiguousarray(
        gx.reshape(MC, 128, SNY * ENC).transpose(1, 0, 2).reshape(128, -1))
    gxdt = np.zeros((128, SNY * 6 * 128), np.float32)
    gxg = gx.reshape(MC, 128, SNY, ENC)
    for mc in range(MC):
        rowoff = 64 * (mc // 6)
        for s in range(SNY):
            gxdt[rowoff:rowoff + 64, s * 768 + (mc % 6) * 128:
                 s * 768 + (mc % 6) * 128 + 128] = gxg[mc, :, s, :].T
    ident = np.eye(128, dtype=np.float32)
    selm = np.zeros((128, C), np.float32)
    for c in range(64):
        selm[c, c::64] = 1.0
        selm[64 + c, c::64] = 1.0

    biases = np.zeros((128, 33), np.float32)

    def put(col, vec):
        nch = max(1, len(vec) // 128)
        for i in range(nch):
            seg = vec[i * 128:(i + 1) * 128]
            biases[0:len(seg), col + i] = seg

    put(BC_RZ, (bih + bhh)[0:1024])
    put(BC_HN, bhh[1024:1536])
    put(BC_XN, bih[1024:1536])
    put(BC_RES, resb)
    put(BC_GBHH, gbhh)
    put(BC_PRED, predb)

    return {
        "wemb": bfc(wemb),
        "wx8": wx8,
        "wxn": bfc(wxn),
        "wh8": wh8,
        "whn": bfc(whn),
        "wres": bfc(resW.T),
        "whd": bfc(gWhh.T),
        "wpred": bfc(predW.T),
        "gxd": bfc(gxd), "gxdt": bfc(gxdt),
        "ident": bfc(ident), "selm": bfc(selm),
        "biases": biases,
    }


def kernel(**inputs):
    global _PROGRAM
    if _PROGRAM is None:
        _PROGRAM = _build_program()
    nc = _PROGRAM
    shared = _prep_host(inputs)
    x = np.ascontiguousarray(inputs["x"], dtype=np.float32)
    ones = np.ones((1, R), np.float32)
    in_maps = []
    for c in range(NCORES):
        xs = x[c * BL:(c + 1) * BL]
        m = dict(shared)
        m["x"] = xs
        m["lastrow"] = np.ascontiguousarray(xs[:, -1, :].reshape(1, R))
        m["ones"] = ones
        in_maps.append(m)
    res = run_bass_kernel_spmd(nc, in_maps, list(range(NCORES)))
    out = np.concatenate([res.results[c]["o"] for c in range(NCORES)], axis=0)
    return out.astype(np.float32)
